# revision 54
# baseline (speedup 1.0000x reference)
"""Bipartite 2-layer SAGEConv GNN on 8 Trainium2 NeuronCores.

Strategy:
  - Edges sharded by destination range (core c owns dst rows [S*c, S*(c+1))
    for BOTH directions, so layer-2 lin_r terms stay core-local).
  - Per core+direction, dsts are sorted by degree; edges packed 1 slot per
    dst-row, 128 dst-rows per PSUM block, variable tiles per block
    (schedule = max over cores, so one SPMD program serves all cores).
  - Message gather: dma_gather with a CENTERED table base (idx int16 signed,
    idx = node - N/2) so all 50001 rows are addressable. Tables are bf16
    (256B rows) so the PE segment-sum matmuls run at 1 cycle/row.
  - Segment-sum: PE matmul with a bf16 identity lhsT accumulated in PSUM
    per 128-dst block (no scatter-add races).
  - Layer 2 transform-first: z = x1 @ w2l.T (64 wide) gathered instead of
    x1. Both directions' z live in ONE combined bf16 table [N+1, 128]
    (cols 0:64 = z_u, 64:128 = z_p) -> a single AllGather, and gather rows
    stay at the 256B SWDGE granularity.
  - Degree-permutation undone at DRAM stores via unique-index
    dma_scatter_add; cross-direction lin_r terms staged in f32.
"""
import sys
import numpy as np

sys.path.insert(0, "/opt/trn_rl_repo")

# ---------------- problem dims (hardcoded for the harness) ----------------
N = 50000
E = 800000
F_IN = 128
HID = 256
CLS = 64
NCORES = 8

SEG = 1            # slots per segment (one dst's edges per tile-row)
BPD = 128          # dsts per psum block
CHUNK_TILES = 8    # tiles per gather call (1024 idx = HW SWDGE ring limit)
SCAT_CHUNK = 1024  # rows per r2 unperm-gather call (1024 descs)
SCRATCH = 49152    # SWDGE descriptor carveout bytes (16B/desc)


class CFG:
    def __init__(self, n=N, e=E, center=None):
        self.N = n
        self.E = e
        self.S = n // NCORES          # dst rows per core
        self.CENTER = n // 2 if center is None else center  # gather table base row
        self.ZROW = n                 # zero row index (centered: n - CENTER >= 0)
        self.NB = -(-self.S // BPD)   # blocks per direction
        self.RT = -(-self.S // 128)   # 128-row tiles of the slice
        self.SP = self.RT * 128       # padded rows


# ---------------- host-side edge scheduling ----------------

def _prep_dir(src_g, dst_g, c, cfg):
    """Per-core, per-direction metadata."""
    lo = c * cfg.S
    m = (dst_g >= lo) & (dst_g < lo + cfg.S)
    ls = src_g[m].astype(np.int64)
    ld = (dst_g[m] - lo).astype(np.int64)
    deg = np.bincount(ld, minlength=cfg.S)
    pi = np.argsort(-deg, kind="stable").astype(np.int64)
    order = np.argsort(ld, kind="stable")
    ls_s = ls[order]
    starts = np.zeros(cfg.S + 1, np.int64)
    starts[1:] = np.cumsum(deg)
    degp = np.zeros(cfg.NB * BPD, np.int64)
    degp[: cfg.S] = deg[pi]
    treq = np.maximum(
        1, -(-degp.reshape(cfg.NB, BPD).max(1) // SEG)
    ).astype(np.int64)
    return dict(pi=pi, deg=deg, starts=starts, ls_s=ls_s, degp=degp, treq=treq)


def _n_tiles(T):
    return int(T.sum())


def _build_slots(meta, T, cfg, iperm):
    """Slot array of COMPOSED table positions (iperm[src node id], ZROW for
    dummies) per the shared schedule.

    Also guarantees every gather-call boundary (CHUNK_TILES-aligned and the
    final tile) ends with a slot whose centered index is non-negative (the
    SWDGE ucode trims trailing negatives), swapping slots within the last
    dst row of the block (order within a row is free; the low-degree tail
    rows are mostly ZROW pads, which sit at position >= CENTER).
    """
    pi, deg, starts, ls_s = meta["pi"], meta["deg"], meta["starts"], meta["ls_s"]
    total_tiles = int(T.sum())
    out = np.full((total_tiles, BPD, SEG), cfg.ZROW, np.int64)
    row_of_tile = np.zeros(total_tiles, np.int64)   # block index per tile
    t0 = 0
    blk_start = {}
    for b in range(cfg.NB):
        tb = int(T[b])
        blk_start[b] = t0
        row_of_tile[t0:t0 + tb] = b
        blk = out[t0 : t0 + tb]          # [tb, BPD, SEG]
        for mrow in range(BPD):
            r = BPD * b + mrow
            if r >= cfg.S:
                continue
            D = int(pi[r])
            d = int(deg[D])
            if d == 0:
                continue
            vals = np.full(tb * SEG, cfg.ZROW, np.int64)
            vals[:d] = ls_s[starts[D] : starts[D] + d]
            blk[:, mrow, :] = vals.reshape(tb, SEG)
        t0 += tb

    out = iperm[out]                     # natural node ids -> table positions

    tails = list(range(CHUNK_TILES - 1, total_tiles, CHUNK_TILES))
    if (total_tiles - 1) not in tails:
        tails.append(total_tiles - 1)
    byblk = {}
    for tg in tails:
        byblk.setdefault(int(row_of_tile[tg]), []).append(tg)
    for b, tgs in byblk.items():
        tb = int(T[b])
        blk = out[blk_start[b] : blk_start[b] + tb]
        jlasts = [(tg - blk_start[b]) * SEG + SEG - 1 for tg in tgs]
        row = blk[:, BPD - 1, :].reshape(-1).copy()
        boundary = set(jlasts)
        for jl in jlasts:
            if row[jl] >= cfg.CENTER:
                continue
            done = False
            for j in range(len(row) - 1, -1, -1):
                if row[j] >= cfg.CENTER and j not in boundary:
                    row[jl], row[j] = row[j], row[jl]
                    done = True
                    break
            assert done, "no non-negative slot available for chunk tail"
        blk[:, BPD - 1, :] = row.reshape(tb, SEG)
    # verify: the last slot of every gather call is non-negative
    flat = out.reshape(total_tiles * 128)
    for tg in tails:
        assert flat[tg * 128 + 127] >= cfg.CENTER, tg
    return out.reshape(total_tiles, 128)


def _wrap16(idx16):
    """[n] int16 -> [128, n/16]: idx i at partition i%16, col i//16, x8 replicas."""
    n = len(idx16)
    assert n % 16 == 0
    return np.tile(idx16.reshape(n // 16, 16).T, (8, 1)).astype(np.int16)


def _pad_idx(idx, ntot):
    out = np.full(ntot, -1, np.int64)
    out[: len(idx)] = idx
    return out


def _bf16(a):
    import ml_dtypes
    return np.asarray(a).astype(ml_dtypes.bfloat16)


def _prep_all(inputs, cfg):
    """Host prep: per-core in_maps + the shared schedule."""
    x_user = np.asarray(inputs["x_user"], np.float32)
    x_product = np.asarray(inputs["x_product"], np.float32)
    ei = np.asarray(inputs["edge_index"]).astype(np.int64)
    u, p = ei[0], ei[1]

    metaA = [_prep_dir(u, p, c, cfg) for c in range(NCORES)]  # dst = p, src = u
    metaB = [_prep_dir(p, u, c, cfg) for c in range(NCORES)]  # dst = u, src = p

    TA = np.max([m["treq"] for m in metaA], axis=0)
    TB = np.max([m["treq"] for m in metaB], axis=0)

    # global degree-sorted layout: table position c*S + j holds natural row
    # c*S + pi_c[j]; iperm maps natural node id -> table position (N -> N).
    def perms_of(metas):
        perm = np.empty(cfg.N + 1, np.int64)
        iperm = np.empty(cfg.N + 1, np.int64)
        for c in range(NCORES):
            rows = c * cfg.S + metas[c]["pi"]
            perm[c * cfg.S : (c + 1) * cfg.S] = rows
            iperm[rows] = c * cfg.S + np.arange(cfg.S)
        perm[cfg.N] = cfg.N
        iperm[cfg.N] = cfg.N
        return perm, iperm

    permA, ipermA = perms_of(metaA)
    permB, ipermB = perms_of(metaB)

    def tab(x, perm):
        t = np.zeros((cfg.N + 1, F_IN), np.float32)
        t[: cfg.N] = x[perm[: cfg.N]]
        return _bf16(t)

    xu_tab, xp_tab = tab(x_user, permA), tab(x_product, permB)

    w = {k: np.asarray(v, np.float32) for k, v in inputs.items()
         if k.startswith(("w_", "b_"))}
    shared = {
        "xu_tab": xu_tab, "xp_tab": xp_tab,
        "wu1lT": _bf16(np.ascontiguousarray(w["w_u1_l"].T)),
        "wu1rT": _bf16(np.ascontiguousarray(w["w_u1_r"].T)),
        "wp1lT": _bf16(np.ascontiguousarray(w["w_p1_l"].T)),
        "wp1rT": _bf16(np.ascontiguousarray(w["w_p1_r"].T)),
        "wu2lT": _bf16(np.ascontiguousarray(w["w_u2_l"].T)),
        "wu2rT": _bf16(np.ascontiguousarray(w["w_u2_r"].T)),
        "wp2lT": _bf16(np.ascontiguousarray(w["w_p2_l"].T)),
        "wp2rT": _bf16(np.ascontiguousarray(w["w_p2_r"].T)),
        "bu1": np.ascontiguousarray(w["b_u1"].reshape(2, 128).T),
        "bp1": np.ascontiguousarray(w["b_p1"].reshape(2, 128).T),
        "bu2": np.ascontiguousarray(w["b_u2"].reshape(CLS, 1)),
        "bp2": np.ascontiguousarray(w["b_p2"].reshape(CLS, 1)),
        "ident": _bf16(np.eye(128, dtype=np.float32)),
    }

    in_maps = []
    for c in range(NCORES):
        d = dict(shared)
        for tag, meta, xsrc, iperm in (("A", metaA[c], x_product, ipermA),
                                       ("B", metaB[c], x_user, ipermB)):
            T = TA if tag == "A" else TB
            slots = _build_slots(meta, T, cfg, iperm)   # composed positions
            # pad the slot array to a whole number of chunks
            nt = slots.shape[0]
            ntp = -(-nt // CHUNK_TILES) * CHUNK_TILES
            slp = np.full((ntp, 128), cfg.ZROW, np.int64)
            slp[:nt] = slots
            d[f"gidx{tag}"] = _wrap16(
                (slp.reshape(-1) - cfg.CENTER).astype(np.int16))
            pi = meta["pi"]
            # r2 stage is written contiguously in the OTHER direction's
            # dst order; this direction reads it at composed positions.
            opi = (metaB[c] if tag == "A" else metaA[c])["pi"]
            ilocal = np.empty(cfg.S, np.int64)
            ilocal[opi] = np.arange(cfg.S)
            d[f"unperm{tag}"] = _wrap16(
                _pad_idx(ilocal[pi], cfg.SP).astype(np.int16))
            invc = np.zeros(cfg.SP, np.float32)
            invc[: cfg.S] = 1.0 / np.maximum(meta["deg"][pi], 1.0)
            d[f"invc{tag}"] = np.ascontiguousarray(
                invc.reshape(cfg.RT, 128).T)
            rows = c * cfg.S + pi
            xd = xsrc[rows]                       # [S, F] permuted dst-rows
            xdT = np.zeros((F_IN, cfg.SP), np.float32)
            xdT[:, : cfg.S] = xd.T
            d[f"xdT{tag}"] = _bf16(xdT)
        in_maps.append(d)

    perms = ([m["pi"] for m in metaA], [m["pi"] for m in metaB])
    return in_maps, TA, TB, perms


# ---------------- device program ----------------

def _dma_gather_narrow(gp, out_ap, in_ap, idxs_ap, num_idxs, num_idxs_reg,
                       elem_size, elem_step):
    """DRAM-source non-transpose dma_gather with a sub-256B elem_size.

    Mirrors BassGpSimd.dma_gather's DRAM path but skips its
    `elem_size_bytes % 256 == 0` assert (that granularity only binds the
    transpose network and the STRIDE encoding; the per-descriptor payload
    can be smaller). The row stride (elem_step) must still be a multiple
    of 256 bytes.
    """
    import concourse.mybir as mybir
    from concourse.bass import MemorySpace
    from concourse import ap_utils

    assert idxs_ap.dtype == mybir.dt.int16
    assert in_ap.dtype == out_ap.dtype
    assert in_ap.space == MemorySpace.DRAM
    assert idxs_ap.space == MemorySpace.SBUF
    assert out_ap.space == MemorySpace.SBUF
    assert ap_utils.ap_is_contiguous(out_ap.ap[1:])
    assert ap_utils.ap_is_contiguous(idxs_ap.ap[1:])
    assert in_ap.ap[-1][1] == out_ap.ap[-1][1] == elem_size
    assert out_ap.ap[0][1] * out_ap.ap[1][1] == ((num_idxs + 127) // 128) * 128
    assert in_ap.ap[0][0] == elem_step
    stride_bytes = elem_step * mybir.dt.size(in_ap.dtype)
    assert stride_bytes % 256 == 0 and stride_bytes // 256 < 256
    _in_ap = gp.lower_ap_dma(in_ap, for_custom_bir_dma=True)
    return gp.add_instruction(
        mybir.InstDMAGatherAnt(
            name=gp.bass.get_next_instruction_name(),
            ins=[*_in_ap, gp.lower_ap(idxs_ap),
                 gp.lower_val_access(gp.to_reg(num_idxs_reg))],
            outs=[gp.lower_ap(out_ap)],
            transpose=False,
            num_idxs=num_idxs,
            elem_size=elem_size,
            stride_bytes_256=stride_bytes // 256,
            gen_mode=0,
            single_packet=True,
            queue_num=0,
            sbuf_tokens_per_rank=0,
            sbuf_free_dim_per_rank=0,
            sbuf_free_dim_pad_per_rank=0,
            sbuf_byte_offset=0,
        )
    )


def _build_nc(cfg, TA, TB, local_mode=False):
    import concourse.bacc as bacc
    import concourse.mybir as mybir
    from concourse.tile import TileContext

    f32, bf16, i16 = mybir.dt.float32, mybir.dt.bfloat16, mybir.dt.int16
    AF = mybir.ActivationFunctionType
    ALU = mybir.AluOpType

    nc = bacc.Bacc(None, target_bir_lowering=False, num_devices=NCORES,
                   dynamic_dma_scratch_size=SCRATCH, num_swdge_queues=1)

    S, SP, RT, NB, CENTER = cfg.S, cfg.SP, cfg.RT, cfg.NB, cfg.CENTER

    ntA = _n_tiles(TA)
    ntB = _n_tiles(TB)

    def colsA():
        return -(-ntA // CHUNK_TILES) * CHUNK_TILES * 8
    def colsB():
        return -(-ntB // CHUNK_TILES) * CHUNK_TILES * 8

    # ---- DRAM declarations ----
    t_xu = nc.dram_tensor("xu_tab", [cfg.N + 1, F_IN], bf16, kind="ExternalInput")
    t_xp = nc.dram_tensor("xp_tab", [cfg.N + 1, F_IN], bf16, kind="ExternalInput")
    tw = {}
    for k in ["wu1lT", "wu1rT", "wp1lT", "wp1rT"]:
        tw[k] = nc.dram_tensor(k, [F_IN, HID], bf16, kind="ExternalInput")
    for k in ["wu2lT", "wu2rT", "wp2lT", "wp2rT"]:
        tw[k] = nc.dram_tensor(k, [HID, CLS], bf16, kind="ExternalInput")
    for k in ["bu1", "bp1"]:
        tw[k] = nc.dram_tensor(k, [128, 2], f32, kind="ExternalInput")
    for k in ["bu2", "bp2"]:
        tw[k] = nc.dram_tensor(k, [CLS, 1], f32, kind="ExternalInput")
    t_ident = nc.dram_tensor("ident", [128, 128], bf16, kind="ExternalInput")
    t_gidxA = nc.dram_tensor("gidxA", [128, colsA()], i16, kind="ExternalInput")
    t_gidxB = nc.dram_tensor("gidxB", [128, colsB()], i16, kind="ExternalInput")
    t_unpA = nc.dram_tensor("unpermA", [128, SP // 16], i16, kind="ExternalInput")
    t_unpB = nc.dram_tensor("unpermB", [128, SP // 16], i16, kind="ExternalInput")
    t_invcA = nc.dram_tensor("invcA", [128, RT], f32, kind="ExternalInput")
    t_invcB = nc.dram_tensor("invcB", [128, RT], f32, kind="ExternalInput")
    t_xdTA = nc.dram_tensor("xdTA", [F_IN, SP], bf16, kind="ExternalInput")
    t_xdTB = nc.dram_tensor("xdTB", [F_IN, SP], bf16, kind="ExternalInput")

    t_xu2 = nc.dram_tensor("xu2", [SP, CLS], f32, kind="ExternalOutput")
    t_xp2 = nc.dram_tensor("xp2", [SP, CLS], f32, kind="ExternalOutput")

    st_zu = nc.dram_tensor("zu_stage", [SP, 2 * CLS], bf16)
    st_zp = nc.dram_tensor("zp_stage", [SP, 2 * CLS], bf16)
    st_r2A = nc.dram_tensor("r2A_stage", [SP, 2 * CLS], bf16)
    st_r2B = nc.dram_tensor("r2B_stage", [SP, 2 * CLS], bf16)
    aspace = "Local" if local_mode else "Shared"
    t_zuf = nc.dram_tensor("zu_full", [cfg.N + 1, 2 * CLS], bf16,
                           addr_space=aspace)
    t_zpf = nc.dram_tensor("zp_full", [cfg.N + 1, 2 * CLS], bf16,
                           addr_space=aspace)

    with TileContext(nc) as tc:
        # ---- persistent SBUF ----
        with tc.tile_pool(name="persist", bufs=1) as pp:
            sb_ident = pp.tile([128, 128], bf16)
            sb_gidxA = pp.tile([128, colsA()], i16)
            sb_gidxB = pp.tile([128, colsB()], i16)
            sb_w = {}
            for k in ["wu1lT", "wu1rT", "wp1lT", "wp1rT"]:
                sb_w[k] = pp.tile([F_IN, HID], bf16, tag=k, name=k)
            for k in ["wu2lT", "wu2rT", "wp2lT", "wp2rT"]:
                sb_w[k] = pp.tile([128, 2, CLS], bf16, tag=k, name=k)
            for k in ["bu1", "bp1"]:
                sb_w[k] = pp.tile([128, 2], f32, tag=k, name=k)
            b2 = {}
            for k in ["bu2", "bp2"]:
                b2[k] = pp.tile([128, 1], f32, tag=k, name=k)
            sb_invcA = pp.tile([128, RT], f32)
            sb_invcB = pp.tile([128, RT], f32)
            sb_unpA = pp.tile([128, SP // 16], i16)
            sb_unpB = pp.tile([128, SP // 16], i16)

            nc.sync.dma_start(out=sb_ident[:], in_=t_ident[:])
            # load idx tables in chunks so the first gathers start early
            for t, sb in ((t_gidxA, sb_gidxA), (t_gidxB, sb_gidxB)):
                ncols = t.shape[1]
                step = -(-ncols // 4)
                for c0 in range(0, ncols, step):
                    c1 = min(c0 + step, ncols)
                    nc.sync.dma_start(out=sb[:, c0:c1], in_=t[:, c0:c1])
            for k, t in tw.items():
                if k in ("bu2", "bp2"):
                    nc.sync.dma_start(out=b2[k][64:64 + CLS, :], in_=t[:])
                elif k in ("wu2lT", "wu2rT", "wp2lT", "wp2rT"):
                    nc.sync.dma_start(
                        out=sb_w[k][:],
                        in_=t.rearrange("(k p) c -> p k c", p=128)[:])
                else:
                    nc.sync.dma_start(out=sb_w[k][:], in_=t[:])
            nc.sync.dma_start(out=sb_invcA[:], in_=t_invcA[:])
            nc.sync.dma_start(out=sb_invcB[:], in_=t_invcB[:])
            nc.sync.dma_start(out=sb_unpA[:], in_=t_unpA[:])
            nc.sync.dma_start(out=sb_unpB[:], in_=t_unpB[:])

            # zero rows of the z tables (pad slots gather them)
            with tc.tile_pool(name="zpool", bufs=1) as zp:
                ztb = zp.tile([1, 2 * CLS], bf16)
                nc.vector.memset(ztb[:], 0.0)
                nc.sync.dma_start(out=t_zuf[cfg.N:cfg.N + 1, :], in_=ztb[:])
                nc.sync.dma_start(out=t_zpf[cfg.N:cfg.N + 1, :], in_=ztb[:])

            # ================= aggregation pass emitter =================
            def agg_gen(mp, ap, gidx_sb, T, table_ap, elem, estep, agg_sb,
                        label, scale_sb):
                """Gather bf16 rows (elem wide, estep apart), segment-sum
                into agg_sb[:, b, :] per 128-dst block, scaling the PSUM
                drain by scale_sb[:, b] (the 1/deg mean factor). Yields the
                block index after each block's drain so a consumer pass can
                be lag-interleaved. mp (the message pool) is shared across
                passes so pass boundaries don't serialize on SBUF reuse."""
                ntiles = _n_tiles(T)
                if True:
                    msgs = {}

                    def chunk_of(tg):
                        ch = tg // CHUNK_TILES
                        if ch not in msgs:
                            t0c = ch * CHUNK_TILES
                            ct = min(CHUNK_TILES, ntiles - t0c)
                            m = mp.tile([128, CHUNK_TILES, elem], bf16,
                                        tag="msg", name=f"msg{label}_{ch}")
                            if (elem * 2) % 256 == 0:
                                nc.gpsimd.dma_gather(
                                    m[:, :ct, :], table_ap,
                                    gidx_sb[:, 8 * t0c:8 * t0c + 8 * ct],
                                    ct * 128, ct * 128, elem,
                                    elem_step=estep)
                            else:
                                _dma_gather_narrow(
                                    nc.gpsimd, m[:, :ct, :], table_ap,
                                    gidx_sb[:, 8 * t0c:8 * t0c + 8 * ct],
                                    ct * 128, ct * 128, elem, estep)
                            msgs[ch] = m
                        return msgs[ch]

                    tg = 0
                    for b in range(cfg.NB):
                        ps = ap.tile([128, elem], f32, tag="ps",
                                     name=f"ps{label}_{b}")
                        for k in range(int(T[b])):
                            m = chunk_of(tg)
                            nc.tensor.matmul(
                                ps[:], sb_ident[:],
                                m[:, tg % CHUNK_TILES, :],
                                start=(k == 0), stop=(k == int(T[b]) - 1))
                            tg += 1
                        nc.vector.tensor_scalar_mul(
                            agg_sb[:, b, :], ps[:], scale_sb[:, b:b + 1])
                        yield b

            # ================= phase-3 emitter (per direction) =================
            # consumes agg (row-major, permuted, bf16), xdT; produces z + r2_other
            # Yields the last agg block needed BEFORE emitting each group, so
            # the driver can emit it lagged behind the producing agg pass.
            def phase3_gen(pools, agg_sb, xdT_t, wl, wr, b1k, w2l, w2r_o,
                           b2_o, st_z, st_r2o, unp_sb, label):
                p3, p3w, psT, psG, psZ = pools
                if True:
                    xdT = p3.tile([F_IN, SP], bf16, tag="xdT")
                    nc.sync.dma_start(out=xdT[:], in_=xdT_t[:])
                    x1T = p3.tile([128, 2, SP], bf16, tag="x1T")
                    zrows = p3.tile([128, RT, CLS], bf16, tag="zrows")
                    r2rows = p3.tile([128, RT, CLS], bf16, tag="r2rows")
                    ngr = -(-RT // 4)
                    for g in range(ngr):
                        jj0 = 4 * g
                        njj = min(4, RT - jj0)
                        yield jj0 + njj - 1   # need agg blocks up to this
                        rg = njj * 128
                        aT = p3w.tile([128, 512], bf16, tag="aT")
                        for q in range(njj):
                            pt = psT.tile([128, 128], bf16, tag="pt")
                            nc.tensor.transpose(pt[:], agg_sb[:, jj0 + q, :],
                                                sb_ident[:])
                            nc.vector.tensor_copy(
                                aT[:, 128 * q:128 * q + 128], pt[:])
                        c0 = 512 * g
                        for h in range(2):
                            po = psG.tile([128, 512], f32, tag="po")
                            nc.tensor.matmul(
                                po[:, :rg], wl[:, 128 * h:128 * h + 128],
                                aT[:, :rg], start=True, stop=False)
                            nc.tensor.matmul(
                                po[:, :rg], wr[:, 128 * h:128 * h + 128],
                                xdT[:, c0:c0 + rg], start=False, stop=True)
                            nc.scalar.activation(
                                x1T[:, h, c0:c0 + rg], po[:, :rg], AF.Relu,
                                bias=b1k[:, h:h + 1])
                        pz = psZ.tile([128, 512], f32, tag="pz")
                        for h in range(2):
                            nc.tensor.matmul(
                                pz[0:CLS, :rg], w2l[:, h, :],
                                x1T[:, h, c0:c0 + rg],
                                start=(h == 0), stop=(h == 1))
                        for h in range(2):
                            nc.tensor.matmul(
                                pz[64:64 + CLS, :rg], w2r_o[:, h, :],
                                x1T[:, h, c0:c0 + rg],
                                start=(h == 0), stop=(h == 1))
                        zr2 = p3w.tile([128, 512], bf16, tag="zr2")
                        nc.vector.tensor_copy(zr2[0:CLS, :rg], pz[0:CLS, :rg])
                        nc.vector.tensor_scalar_add(
                            zr2[64:64 + CLS, :rg], pz[64:64 + CLS, :rg],
                            b2_o[64:64 + CLS, 0:1])
                        for q in range(njj):
                            pb = psT.tile([128, 128], bf16, tag="pt")
                            nc.tensor.transpose(
                                pb[:, :], zr2[:, 128 * q:128 * q + 128],
                                sb_ident[:])
                            nc.vector.tensor_copy(
                                zrows[:, jj0 + q, :], pb[:, 0:CLS])
                            nc.vector.tensor_copy(
                                r2rows[:, jj0 + q, :], pb[:, 64:64 + CLS])
                        # stage writes per group: overlap later groups'
                        # compute and shrink the pre-collective tail
                        gsl = slice(jj0, jj0 + njj)
                        nc.sync.dma_start(
                            out=st_z.rearrange("(q p) f -> p q f", p=128)
                            [:, gsl, 0:CLS],
                            in_=zrows[:, gsl, :])
                        nc.sync.dma_start(
                            out=st_r2o.rearrange("(q p) f -> p q f", p=128)
                            [:, gsl, 0:CLS],
                            in_=r2rows[:, gsl, :])


            # ================= phase-7 emitter =================
            # Yields (a) once after the r2 prefetch gathers (no agg2 dep),
            # then (b) the last agg2 block needed before each compute group.
            def phase7_gen(p7, agg2_sb, st_r2, unp_sb, t_out, label):
                if True:
                    r2r = p7.tile([128, RT, CLS], bf16, tag="r2r")
                    for k0 in range(0, SP, SCAT_CHUNK):
                        nv = min(SCAT_CHUNK, S - k0)
                        if nv <= 0:
                            break
                        kt = min(SCAT_CHUNK, SP - k0) // 128
                        _dma_gather_narrow(
                            nc.gpsimd,
                            r2r[:, k0 // 128:k0 // 128 + kt, :],
                            st_r2[:, 0:CLS],
                            unp_sb[:, k0 // 16:(k0 + 128 * kt) // 16],
                            kt * 128, min(nv, kt * 128), CLS, 2 * CLS)
                    outt = p7.tile([128, RT, CLS], f32, tag="outt")
                    t_out_r = t_out.rearrange("(q p) f -> p q f", p=128)
                    for g in range(-(-RT // 8)):
                        q0 = 8 * g
                        nq = min(8, RT - q0)
                        yield q0 + nq - 1
                        for q in range(q0, q0 + nq):
                            nc.vector.tensor_tensor(
                                out=outt[:, q, :], in0=agg2_sb[:, q, :],
                                in1=r2r[:, q, :], op=ALU.add)
                        # output stays in this direction's dst order; the
                        # host un-permutes after the run
                        nc.sync.dma_start(
                            out=t_out_r[:, q0:q0 + nq, :],
                            in_=outt[:, q0:q0 + nq, :])

            # ====== driver: lag-interleave a consumer gen behind agg_gen ======
            def drive_gen(agen, cgen):
                """Advance cgen whenever agen has produced the blocks the
                next consumer group needs; emission order stays consistent
                with true dependencies so in-order engine queues can't
                deadlock. cgen is primed only after agen's first yield so
                its tile pools nest inside agen's (LIFO release order).
                Yields after each producer step so two drives can be
                co-scheduled."""
                primed = False
                need = None
                for b in agen:
                    if not primed:
                        need = next(cgen, None)
                        primed = True
                    while need is not None and need <= b:
                        need = next(cgen, None)
                    yield
                if not primed:
                    need = next(cgen, None)
                while need is not None:
                    need = next(cgen, None)

            def drive(agen, cgen):
                for _ in drive_gen(agen, cgen):
                    pass

            # ================= emit the whole program =================
            import os as _os
            PARTS = set((_os.environ.get("KERNEL_PARTS") or
                         "agg1,p3,cc,agg2,p7").split(","))

            def maybe(gen, part):
                return gen if part in PARTS else iter(())

            def cc(st, t_full):
                """Publish one direction's z table as soon as its producer
                phase-3 is done (contiguous AllGather; the junk upper half
                of each row rides along)."""
                if "cc" not in PARTS:
                    return
                if local_mode:
                    nc.sync.dma_start(out=t_full[0:S, :], in_=st[0:S, :])
                else:
                    nc.gpsimd.collective_compute(
                        "AllGather", mybir.AluOpType.bypass,
                        replica_groups=[list(range(NCORES))],
                        ins=[st[0:S, :]], outs=[t_full[0:cfg.N, :]])

            def p3_pools(label):
                return (tc.tile_pool(name=f"p3{label}", bufs=1),
                        tc.tile_pool(name=f"p3w{label}", bufs=2),
                        tc.tile_pool(name=f"psT{label}", bufs=1,
                                     space="PSUM"),
                        tc.tile_pool(name=f"psG{label}", bufs=1,
                                     space="PSUM"),
                        tc.tile_pool(name=f"psZ{label}", bufs=1,
                                     space="PSUM"))

            from contextlib import ExitStack

            with tc.tile_pool(name="msgpool", bufs=8) as mpool, \
                 tc.tile_pool(name="msgpool2", bufs=8) as mpool2, \
                 tc.tile_pool(name="aggpool", bufs=2) as aggpool:
                aggA = aggpool.tile([128, NB, F_IN], bf16, tag="agg",
                                    name="aggA")
                with ExitStack() as es:
                    apA = es.enter_context(
                        tc.tile_pool(name="aggpsA", bufs=2, space="PSUM"))
                    pls = tuple(es.enter_context(p) for p in p3_pools("A"))
                    drive(maybe(agg_gen(mpool, apA, sb_gidxA, TA,
                                        t_xu[CENTER:, :], F_IN, F_IN, aggA,
                                        "A", sb_invcA), "agg1"),
                          maybe(phase3_gen(
                              pls, aggA, t_xdTA, sb_w["wu1lT"],
                              sb_w["wu1rT"], sb_w["bu1"], sb_w["wu2lT"],
                              sb_w["wp2rT"], b2["bp2"], st_zu, st_r2B,
                              sb_unpA, "A"), "p3"))
                cc(st_zu, t_zuf)

                # merged pass: L1-B (DMA-bound) runs alongside L2-A
                # (Pool-gen-bound) so both engines stay saturated. p7A is
                # primed only once p3B has emitted its r2 stage write.
                agg2A = aggpool.tile([128, NB, CLS], f32, tag="agg",
                                     name="agg2A")
                aggB = aggpool.tile([128, NB, F_IN], bf16, tag="agg",
                                    name="aggB")
                with ExitStack() as es:
                    apB = es.enter_context(
                        tc.tile_pool(name="aggpsB", bufs=2, space="PSUM"))
                    plsB = tuple(es.enter_context(p) for p in p3_pools("B"))
                    apA2 = es.enter_context(
                        tc.tile_pool(name="aggpsA2", bufs=2, space="PSUM"))
                    p7pA = es.enter_context(
                        tc.tile_pool(name="p7A", bufs=1))
                    g1 = drive_gen(
                        maybe(agg_gen(mpool, apB, sb_gidxB, TB,
                                      t_xp[CENTER:, :], F_IN, F_IN, aggB,
                                      "B", sb_invcB), "agg1"),
                        maybe(phase3_gen(
                            plsB, aggB, t_xdTB, sb_w["wp1lT"],
                            sb_w["wp1rT"], sb_w["bp1"], sb_w["wp2lT"],
                            sb_w["wu2rT"], b2["bu2"], st_zp, st_r2A,
                            sb_unpB, "B"), "p3"))
                    g2 = maybe(agg_gen(mpool2, apA2, sb_gidxA, TA,
                                       t_zuf[CENTER:, 0:CLS], CLS, 2 * CLS,
                                       agg2A, "A2", sb_invcA), "agg2")
                    p7a = maybe(phase7_gen(p7pA, agg2A, st_r2A, sb_unpA,
                                           t_xu2, "A"), "p7")
                    a1 = a2 = True
                    done2 = -1
                    need7 = None
                    primed7 = False
                    while a1 or a2:
                        if a1:
                            try:
                                next(g1)
                            except StopIteration:
                                a1 = False
                                cc(st_zp, t_zpf)
                        if a2:
                            b = next(g2, None)
                            if b is None:
                                a2 = False
                            else:
                                done2 = b
                        if not a1 and not primed7:
                            need7 = next(p7a, None)
                            primed7 = True
                        while (primed7 and need7 is not None
                               and need7 <= done2):
                            need7 = next(p7a, None)
                    if not primed7:
                        need7 = next(p7a, None)
                    while need7 is not None:
                        need7 = next(p7a, None)

                agg2B = aggpool.tile([128, NB, CLS], f32, tag="agg",
                                     name="agg2B")
                with ExitStack() as es:
                    apB2 = es.enter_context(
                        tc.tile_pool(name="aggpsB2", bufs=2, space="PSUM"))
                    p7pB = es.enter_context(
                        tc.tile_pool(name="p7B", bufs=1))
                    drive(maybe(agg_gen(mpool2, apB2, sb_gidxB, TB,
                                        t_zpf[CENTER:, 0:CLS], CLS, 2 * CLS,
                                        agg2B, "B2", sb_invcB), "agg2"),
                          maybe(phase7_gen(p7pB, agg2B, st_r2B, sb_unpB,
                                           t_xp2, "B"), "p7"))

    nc.finalize()
    return nc


def build(inputs, cfg=None, local_mode=False):
    cfg = cfg or CFG()
    in_maps, TA, TB, perms = _prep_all(inputs, cfg)
    nc = _build_nc(cfg, TA, TB, local_mode=local_mode)
    return nc, in_maps, perms


def unpermute(results, perms, cfg):
    """results[c][name] rows are in per-core degree-sorted order; undo."""
    pisA, pisB = perms
    xu2 = np.empty((cfg.N, CLS), np.float32)
    xp2 = np.empty((cfg.N, CLS), np.float32)
    for c in range(NCORES):
        xu2[c * cfg.S + pisA[c]] = results[c]["xu2"][: cfg.S]
        xp2[c * cfg.S + pisB[c]] = results[c]["xp2"][: cfg.S]
    return xu2, xp2


def kernel(**inputs):
    from concourse.bass_utils import run_bass_kernel_spmd

    cfg = CFG()
    nc, in_maps, perms = build(inputs, cfg)
    res = run_bass_kernel_spmd(nc, in_maps, list(range(NCORES)))
    return unpermute(res.results, perms, cfg)
